# revision 8
# baseline (speedup 1.0000x reference)
"""CenterLoss kernel for Trainium2. BISECT PHASE B: bf16 + TTR/halves compute,
no PE reduce, [P,4] out; DMAs unchanged (sync + gpsimd, single features DMA,
separate gather dest tiles)."""

import numpy as np
import ml_dtypes

import concourse.bacc as bacc
import concourse.bass as bass
import concourse.mybir as mybir
from concourse.bass_utils import run_bass_kernel_spmd
from concourse.tile import TileContext

N = 8192
D = 512
C = 10000
NCORES = 8
N_LOC = N // NCORES  # 1024 rows per core
P = 128
NTILES = N_LOC // P  # 8 tiles of 128 rows
COLS = NTILES * D  # 4096
NH = 2
HC = COLS // NH  # 2048
XA = 512  # ACT's share per half


def build_nc() -> bass.Bass:
    nc = bacc.Bacc(
        dynamic_dma_scratch_size=98304,
        enable_partition_id=False,
        enable_asserts=False,
    )

    feats = nc.dram_tensor(
        "features_t", [P, COLS], mybir.dt.bfloat16, kind="ExternalInput"
    )
    centers = nc.dram_tensor("centers", [C, D], mybir.dt.bfloat16, kind="ExternalInput")
    labels = nc.dram_tensor(
        "labels_t", [P, NTILES], mybir.dt.int32, kind="ExternalInput"
    )
    out = nc.dram_tensor("partial", [P, 2 * NH], mybir.dt.float32, kind="ExternalOutput")

    with TileContext(nc) as tc:
        with tc.tile_pool(name="sbuf", bufs=1) as pool:
            lab_tile = pool.tile([P, NTILES], mybir.dt.int32)
            nc.sync.dma_start(out=lab_tile[:], in_=labels[:])

            ftile = pool.tile([P, COLS], mybir.dt.bfloat16)
            nc.sync.dma_start(out=ftile[:], in_=feats[:])

            gats = [
                pool.tile([P, D], mybir.dt.bfloat16, name=f"gat{j}", tag=f"gat{j}")
                for j in range(NTILES)
            ]
            for j in range(NTILES):
                nc.gpsimd.indirect_dma_start(
                    out=gats[j][:],
                    out_offset=None,
                    in_=centers[:],
                    in_offset=bass.IndirectOffsetOnAxis(
                        ap=lab_tile[:, j : j + 1], axis=0
                    ),
                )

            diff = pool.tile([P, COLS], mybir.dt.bfloat16)
            for j in range(NTILES):
                js = slice(j * D, (j + 1) * D)
                nc.vector.tensor_tensor(
                    out=diff[:, js],
                    in0=ftile[:, js],
                    in1=gats[j][:],
                    op=mybir.AluOpType.subtract,
                )

            acc = pool.tile([P, 2 * NH], mybir.dt.float32)
            sq = pool.tile([P, HC - XA], mybir.dt.bfloat16)
            for h in range(NH):
                a0 = h * HC
                nc.scalar.activation(
                    out=diff[:, a0 : a0 + XA],
                    in_=diff[:, a0 : a0 + XA],
                    func=mybir.ActivationFunctionType.Square,
                    accum_out=acc[:, h : h + 1],
                )
                nc.vector.tensor_tensor(
                    out=sq[:],
                    in0=diff[:, a0 + XA : a0 + HC],
                    in1=diff[:, a0 + XA : a0 + HC],
                    op=mybir.AluOpType.mult,
                )
                nc.vector.tensor_reduce(
                    out=acc[:, NH + h : NH + h + 1],
                    in_=sq[:],
                    axis=mybir.AxisListType.X,
                    op=mybir.AluOpType.add,
                )

            nc.sync.dma_start(out=out[:], in_=acc[:])

    nc.finalize()
    return nc


_NC_CACHE: list = []


def get_nc() -> bass.Bass:
    if not _NC_CACHE:
        _NC_CACHE.append(build_nc())
    return _NC_CACHE[0]


def prepare_in_maps(features, labels, centers):
    features = np.asarray(features, dtype=np.float32).astype(ml_dtypes.bfloat16)
    centers = np.ascontiguousarray(
        np.asarray(centers, dtype=np.float32).astype(ml_dtypes.bfloat16)
    )
    labels32 = np.asarray(labels).astype(np.int32)

    in_maps = []
    for c in range(NCORES):
        f = features[c * N_LOC : (c + 1) * N_LOC]
        lab = labels32[c * N_LOC : (c + 1) * N_LOC]
        f_t = np.ascontiguousarray(
            f.reshape(NTILES, P, D).transpose(1, 0, 2).reshape(P, COLS)
        )
        lab_t = np.ascontiguousarray(lab.reshape(NTILES, P).T)
        in_maps.append({"features_t": f_t, "centers": centers, "labels_t": lab_t})
    return in_maps


def collect(results):
    total = sum(
        float(np.asarray(r["partial"], dtype=np.float64).sum()) for r in results
    )
    return np.float32(total / N)


def kernel(features, labels, centers):
    nc = get_nc()
    in_maps = prepare_in_maps(features, labels, centers)
    results = run_bass_kernel_spmd(nc, in_maps, list(range(NCORES))).results
    return collect(results)


# revision 9
# speedup vs baseline: 1.1975x; 1.1975x over previous
"""CenterLoss kernel for Trainium2. PHASE B3: bf16, single [128,8]-offset
indirect gather (1024 descriptors in one SWDGE instruction), DVE subtracts,
ACT squares with fused accumulation, host-side final sum."""

import numpy as np
import ml_dtypes

import concourse.bacc as bacc
import concourse.bass as bass
import concourse.mybir as mybir
from concourse.bass_utils import run_bass_kernel_spmd
from concourse.tile import TileContext

N = 8192
D = 512
C = 10000
NCORES = 8
N_LOC = N // NCORES  # 1024 rows per core
P = 128
NTILES = N_LOC // P  # 8 tiles of 128 rows
COLS = NTILES * D  # 4096
NH = 2
HC = COLS // NH  # 2048


def build_nc() -> bass.Bass:
    nc = bacc.Bacc(
        dynamic_dma_scratch_size=98304,
        enable_partition_id=False,
        enable_asserts=False,
    )

    feats = nc.dram_tensor(
        "features_t", [P, COLS], mybir.dt.bfloat16, kind="ExternalInput"
    )
    centers = nc.dram_tensor("centers", [C, D], mybir.dt.bfloat16, kind="ExternalInput")
    labels = nc.dram_tensor(
        "labels_t", [P, NTILES], mybir.dt.int32, kind="ExternalInput"
    )
    out = nc.dram_tensor("partial", [P, NH], mybir.dt.float32, kind="ExternalOutput")

    with TileContext(nc) as tc:
        with tc.tile_pool(name="sbuf", bufs=1) as pool:
            lab_tile = pool.tile([P, NTILES], mybir.dt.int32)
            nc.sync.dma_start(out=lab_tile[:], in_=labels[:])

            ftile = pool.tile([P, COLS], mybir.dt.bfloat16)
            nc.sync.dma_start(out=ftile[:], in_=feats[:])

            gat = pool.tile([P, COLS], mybir.dt.bfloat16)
            nc.gpsimd.indirect_dma_start(
                out=gat[:],
                out_offset=None,
                in_=centers[:],
                in_offset=bass.IndirectOffsetOnAxis(ap=lab_tile[:], axis=0),
            )

            diff = pool.tile([P, COLS], mybir.dt.bfloat16)
            for j in range(NTILES):
                js = slice(j * D, (j + 1) * D)
                nc.vector.tensor_tensor(
                    out=diff[:, js],
                    in0=ftile[:, js],
                    in1=gat[:, js],
                    op=mybir.AluOpType.subtract,
                )

            acc = pool.tile([P, NH], mybir.dt.float32)
            for h in range(NH):
                hs = slice(h * HC, (h + 1) * HC)
                nc.scalar.activation(
                    out=diff[:, hs],
                    in_=diff[:, hs],
                    func=mybir.ActivationFunctionType.Square,
                    accum_out=acc[:, h : h + 1],
                )

            nc.sync.dma_start(out=out[:], in_=acc[:])

    nc.finalize()
    return nc


_NC_CACHE: list = []


def get_nc() -> bass.Bass:
    if not _NC_CACHE:
        _NC_CACHE.append(build_nc())
    return _NC_CACHE[0]


def prepare_in_maps(features, labels, centers):
    features = np.asarray(features, dtype=np.float32).astype(ml_dtypes.bfloat16)
    centers = np.ascontiguousarray(
        np.asarray(centers, dtype=np.float32).astype(ml_dtypes.bfloat16)
    )
    labels32 = np.asarray(labels).astype(np.int32)

    in_maps = []
    for c in range(NCORES):
        f = features[c * N_LOC : (c + 1) * N_LOC]
        lab = labels32[c * N_LOC : (c + 1) * N_LOC]
        f_t = np.ascontiguousarray(
            f.reshape(NTILES, P, D).transpose(1, 0, 2).reshape(P, COLS)
        )
        lab_t = np.ascontiguousarray(lab.reshape(NTILES, P).T)
        in_maps.append({"features_t": f_t, "centers": centers, "labels_t": lab_t})
    return in_maps


def collect(results):
    total = sum(
        float(np.asarray(r["partial"], dtype=np.float64).sum()) for r in results
    )
    return np.float32(total / N)


def kernel(features, labels, centers):
    nc = get_nc()
    in_maps = prepare_in_maps(features, labels, centers)
    results = run_bass_kernel_spmd(nc, in_maps, list(range(NCORES))).results
    return collect(results)
